# revision 38
# baseline (speedup 1.0000x reference)
"""Trainium2 Bass kernel for nn_CGCN (curvature-weighted GCN + pair decoder).

Strategy (8 NeuronCores, SPMD):
  - Edges sharded by DESTINATION node: core c owns nodes [c*N/8, (c+1)*N/8)
    and every edge whose col lands there (host bins/sorts; index plumbing
    only). The scatter-add stays core-local - no [N,H] all-reduce.
  - Edge weights ew = Linear(func_k(curvature)) on device via Horner in
    t = exp(-c).
  - Degrees: per-node padded layout [node-on-partition, window x Dmax],
    fp32 vector reduce (exact per-node sums, no long-prefix cancellation).
  - y = dinv * (x @ W) per 128-node window (bf16), AllGathered.
  - Aggregation: per 128-edge chunk, S[p,j] = ew[p] * (dst_rel[p]==j) built
    on DVE; psum[window,H] += S^T @ y_rows on PE. y rows fetched with
    dma_gather (int16 idx), the table split in 4 segments of 2*NPAD rows so
    indices fit int16; chunks are organized in 4 segment planes.
  - x1 = relu(dinv*(agg+y)+b), AllGathered (bf16).
  - Decoder: pairs grouped by (seg(e1), seg(e2)) into 16 padded groups so
    each dma_gather call has a single segment base; host un-permutes the
    output. feat@dec1_w decomposed as e1@(Wa+Wc) + e2@(Wa+Wd) + (e1*e2)@Wb.
"""
import sys

for _p in ("/opt/trn_rl_repo",):
    if _p not in sys.path:
        sys.path.append(_p)

import numpy as np
from contextlib import ExitStack

import concourse.bass as bass
import concourse.tile as tile
from concourse import mybir, bacc
from concourse.masks import make_identity

F32 = mybir.dt.float32
F32R = mybir.dt.float32r
BF16 = mybir.dt.bfloat16
I32 = mybir.dt.int32
I16 = mybir.dt.int16

NCORES = 8
NSEG = 4          # table segments (2 cores' rows each) so idx fits int16


def _wrap16(a):
    """[128, n] per-slot idx array -> dma_gather idx layout [128, n*8] int16.

    Slot (chunk c, partition p) maps to idx-list position i = c*128 + p;
    the ucode reads position i from [p16=i%16, col=c*8 + (i%128)//16],
    partitions replicated x8.
    """
    p128, n = a.shape
    assert p128 == 128
    m = np.zeros((16, n, 8), np.int16)
    for s16 in range(8):
        m[:, :, s16] = a[s16 * 16:(s16 + 1) * 16, :]
    m = m.reshape(16, n * 8)
    return np.tile(m, (8, 1)).copy()


class Dims:
    def __init__(self, N, E, P, FIN, H, D1, CS, D, PG):
        self.N, self.E, self.P = N, E, P
        self.FIN, self.H, self.D1 = FIN, H, D1
        self.CS = list(CS)             # chunks per window per segment plane
        self.D = D                     # max node in-degree
        self.PG = PG                   # decoder chunks per pair-group (even)
        self.NSH = N // NCORES
        self.W = (self.NSH + 127) // 128
        self.NPAD = self.W * 128
        self.SEGR = 2 * self.NPAD      # rows per table segment
        self.NCH = self.W * sum(CS)    # total chunk columns
        self.PB = [self.W * sum(CS[:s]) for s in range(NSEG)]  # plane col base
        self.WD = self.W * D
        self.PPC = P // NCORES
        self.PCH = self.PPC // 128     # real decoder chunks
        self.PCH2 = 16 * PG            # padded decoder chunks (16 groups)
        self.KF = FIN // 128
        self.GBW = 1   # >1 risks SWDGE ring/packet overflow (561-desc calls hang HW)
        self.WH = 2 if self.W % 2 == 0 else 1
        assert self.SEGR <= 32768
        assert PG % 2 == 0


def chunk_params(edge_index, idx, N):
    """Uniform (CS per segment, D, PG) from the data."""
    NSH = N // NCORES
    W = (NSH + 127) // 128
    NPAD = W * 128
    col = np.asarray(edge_index[1], np.int64)
    row = np.asarray(edge_index[0], np.int64)
    order = np.argsort(col, kind="stable")
    col_s, row_s = col[order], row[order]
    D = int(np.bincount(col_s, minlength=N).max())
    seg_of_row = ((row_s // NSH) * NPAD + (row_s % NSH)) // (2 * NPAD)
    CS = [1] * NSEG
    for c in range(NCORES):
        lo, hi = np.searchsorted(col_s, [c * NSH, (c + 1) * NSH])
        wof = (col_s[lo:hi] - c * NSH) >> 7
        sof = seg_of_row[lo:hi]
        cnt = np.bincount(wof * NSEG + sof, minlength=W * NSEG).reshape(W, NSEG)
        for s in range(NSEG):
            CS[s] = max(CS[s], int(np.ceil(cnt[:, s].max() / 128)))
    # decoder groups
    i1 = np.asarray(idx[0], np.int64)
    i2 = np.asarray(idx[1], np.int64)
    g1 = ((i1 // NSH) * NPAD + (i1 % NSH)) // (2 * NPAD)
    g2 = ((i2 // NSH) * NPAD + (i2 % NSH)) // (2 * NPAD)
    PPC = i1.shape[0] // NCORES
    PG = 1
    for c in range(NCORES):
        g = g1[c * PPC:(c + 1) * PPC] * NSEG + g2[c * PPC:(c + 1) * PPC]
        cnt = np.bincount(g, minlength=NSEG * NSEG)
        PG = max(PG, int(np.ceil(cnt.max() / 128)))
    if PG % 2:
        PG += 1
    return CS, D, PG


def preprocess(d, x, curvature, edge_index, idx, b_gcn, lin1_w, lin1_b):
    """Index plumbing: sort/bin/pad edges and pairs, build per-core inputs."""
    row = np.asarray(edge_index[0], dtype=np.int64)
    col = np.asarray(edge_index[1], dtype=np.int64)
    cur = np.asarray(curvature, dtype=np.float32)
    order = np.argsort(col, kind="stable")
    row_s, col_s, cur_s = row[order], col[order], cur[order]
    bounds = np.searchsorted(col_s, np.arange(NCORES + 1) * d.NSH)

    iota_f = np.broadcast_to(np.arange(128, dtype=np.float32), (128, 128)).copy()
    b_mat = np.broadcast_to(np.asarray(b_gcn, np.float32)[None, :], (128, d.H)).copy()
    linw_rep = np.broadcast_to(np.asarray(lin1_w, np.float32)[:, 0][None, :], (128, 10)).copy()
    linb_rep = np.full((128, 1), np.float32(np.asarray(lin1_b).reshape(-1)[0]), np.float32)

    def gidx(nodes):
        return ((nodes // d.NSH) * d.NPAD + (nodes % d.NSH)).astype(np.int64)

    maps = []
    perms = []
    for c in range(NCORES):
        lo, hi = bounds[c], bounds[c + 1]
        colr = (col_s[lo:hi] - c * d.NSH).astype(np.int64)
        grow = gidx(row_s[lo:hi])
        curc = cur_s[lo:hi]
        n_c = colr.shape[0]
        wof = colr >> 7
        sof = grow // d.SEGR
        # order edges by (window, segment) [stable within = by col]
        okey = np.argsort(wof * NSEG + sof, kind="stable")
        colr, grow, curc = colr[okey], grow[okey], curc[okey]
        wof, sof = wof[okey], sof[okey]
        # chunk column for each edge
        cnt = np.bincount(wof * NSEG + sof, minlength=d.W * NSEG).reshape(d.W, NSEG)
        start = np.zeros(d.W * NSEG, np.int64)
        start[1:] = np.cumsum(cnt.reshape(-1))[:-1]
        pos = np.arange(n_c) - start[wof * NSEG + sof]
        ci = np.array(d.PB, np.int64)[sof] + wof * np.array(d.CS, np.int64)[sof] + (pos >> 7)
        p = (pos & 127).astype(np.int64)

        dstf = np.full((128, d.NCH), -1.0, np.float32)
        dstf[p, ci] = (colr & 127).astype(np.float32)
        curb = np.zeros((128, d.NCH), np.float32)
        curb[p, ci] = curc
        ridx = np.zeros((128, d.NCH), np.int64)
        ridx[p, ci] = grow - sof * d.SEGR
        g16 = _wrap16(ridx.astype(np.int16))

        # degree layout (per-node padded)
        colr2 = col_s[lo:hi] - c * d.NSH
        cur2 = cur_s[lo:hi]
        nstart = np.searchsorted(colr2, np.arange(d.NSH))
        dpos = np.arange(n_c) - nstart[colr2]
        pn = (colr2 & 127).astype(np.int64)
        fi = (colr2 >> 7) * d.D + dpos
        curd = np.zeros((128, d.WD), np.float32)
        curd[pn, fi] = cur2
        maskd = np.zeros((128, d.WD), np.float32)
        maskd[pn, fi] = 1.0

        # host-transposed x shard, bf16: xt[p, k*NPAD + n] = x[base+n, k*128+p]
        x_pad = np.zeros((d.NPAD, d.FIN), np.float32)
        x_pad[:d.NSH] = x[c * d.NSH:(c + 1) * d.NSH]
        import ml_dtypes
        xt = np.ascontiguousarray(
            x_pad.reshape(d.NPAD, d.KF, 128).transpose(2, 1, 0)
        ).reshape(128, d.KF * d.NPAD).astype(ml_dtypes.bfloat16)

        # decoder pair groups
        gi1 = gidx(np.asarray(idx[0][c * d.PPC:(c + 1) * d.PPC], np.int64))
        gi2 = gidx(np.asarray(idx[1][c * d.PPC:(c + 1) * d.PPC], np.int64))
        grp = (gi1 // d.SEGR) * NSEG + (gi2 // d.SEGR)
        pkey = np.argsort(grp, kind="stable")
        gcnt = np.bincount(grp, minlength=16)
        gstart = np.zeros(16, np.int64)
        gstart[1:] = np.cumsum(gcnt)[:-1]
        ppos = np.arange(d.PPC) - gstart[grp[pkey]]
        slot = grp[pkey] * d.PG * 128 + ppos        # slot in padded layout
        o1a = np.zeros((128, d.PCH2), np.int64)
        o2a = np.zeros((128, d.PCH2), np.int64)
        o1a[slot & 127, slot >> 7] = gi1[pkey] % d.SEGR
        o2a[slot & 127, slot >> 7] = gi2[pkey] % d.SEGR
        o1g = _wrap16(o1a.astype(np.int16))
        o2g = _wrap16(o2a.astype(np.int16))
        perms.append(np.asarray(slot))              # real pair pkey[j] -> slot[j]
        pk = np.empty(d.PPC, np.int64)
        pk[:] = pkey
        perms[-1] = (pk, np.asarray(slot))

        maps.append(dict(
            xt=xt, curb=curb, dstf=dstf, g16=g16,
            curd=curd, maskd=maskd, o1g=o1g, o2g=o2g,
            iota_f=iota_f, b_mat=b_mat, linw=linw_rep, linb=linb_rep,
        ))
    return maps, perms


def build(d):
    nc = bacc.Bacc("TRN2", target_bir_lowering=False, debug=False,
                   num_devices=NCORES)
    H, D, W = d.H, d.D, d.W

    xt_d = nc.dram_tensor("xt", [128, d.KF * d.NPAD], BF16, kind="ExternalInput")
    curb = nc.dram_tensor("curb", [128, d.NCH], F32, kind="ExternalInput")
    dstf = nc.dram_tensor("dstf", [128, d.NCH], F32, kind="ExternalInput")
    g16 = nc.dram_tensor("g16", [128, d.NCH * 8], I16, kind="ExternalInput")
    curd = nc.dram_tensor("curd", [128, d.WD], F32, kind="ExternalInput")
    maskd = nc.dram_tensor("maskd", [128, d.WD], F32, kind="ExternalInput")
    o1g = nc.dram_tensor("o1g", [128, d.PCH2 * 8], I16, kind="ExternalInput")
    o2g = nc.dram_tensor("o2g", [128, d.PCH2 * 8], I16, kind="ExternalInput")
    iota_f = nc.dram_tensor("iota_f", [128, 128], F32, kind="ExternalInput")
    b_mat = nc.dram_tensor("b_mat", [128, H], F32, kind="ExternalInput")
    linw = nc.dram_tensor("linw", [128, 10], F32, kind="ExternalInput")
    linb = nc.dram_tensor("linb", [128, 1], F32, kind="ExternalInput")
    w_gcn = nc.dram_tensor("w_gcn", [d.FIN, H], F32, kind="ExternalInput")
    dec1_w = nc.dram_tensor("dec1_w", [4 * H, d.D1], F32, kind="ExternalInput")
    dec1_b = nc.dram_tensor("dec1_b", [d.D1], F32, kind="ExternalInput")
    dec2_w = nc.dram_tensor("dec2_w", [d.D1, 1], F32, kind="ExternalInput")
    dec2_b = nc.dram_tensor("dec2_b", [1], F32, kind="ExternalInput")
    out_d = nc.dram_tensor("out_d", [d.PCH2 * 128, 1], F32, kind="ExternalOutput")

    AT = mybir.ActivationFunctionType
    OP = mybir.AluOpType
    MB = d.D1 // 128

    with ExitStack() as ctx:
        tc = ctx.enter_context(tile.TileContext(nc))
        const = ctx.enter_context(tc.tile_pool(name="const", bufs=1))
        sb = ctx.enter_context(tc.tile_pool(name="sb", bufs=3))
        big = ctx.enter_context(tc.tile_pool(name="big", bufs=3))
        gp = ctx.enter_context(tc.tile_pool(name="gp", bufs=5))   # gather tiles
        ps = ctx.enter_context(tc.tile_pool(name="ps", bufs=2, space="PSUM"))
        dr = ctx.enter_context(tc.tile_pool(name="dr", bufs=1, space="DRAM"))

        # ---------- constants ----------
        iota_sb = const.tile([128, 128], F32, tag="iota32")
        nc.sync.dma_start(out=iota_sb[:], in_=iota_f.ap())
        iota_bf = const.tile([128, 128], BF16, tag="iotabf")
        nc.vector.tensor_copy(out=iota_bf[:], in_=iota_sb[:])
        ident = const.tile([128, 128], BF16, tag="ident")
        make_identity(nc, ident[:])
        ident_f = const.tile([128, 128], F32, tag="identf")
        make_identity(nc, ident_f[:])
        bmat_sb = const.tile([128, H], F32, tag="bmat")
        nc.sync.dma_start(out=bmat_sb[:], in_=b_mat.ap())

        linw_sb = const.tile([128, 10], F32, tag="linw")
        nc.sync.dma_start(out=linw_sb[:], in_=linw.ap())
        linb_sb = const.tile([128, 1], F32, tag="linb")
        nc.sync.dma_start(out=linb_sb[:], in_=linb.ap())
        coef = const.tile([128, 10], F32, tag="coef")
        nc.vector.tensor_scalar_mul(coef[:], linw_sb[:], 0.5)
        csum = const.tile([128, 1], F32, tag="csum")
        nc.vector.tensor_reduce(out=csum[:], in_=coef[:], axis=mybir.AxisListType.X,
                                op=OP.add)
        cconst = const.tile([128, 1], F32, tag="cconst")
        nc.vector.tensor_add(cconst[:], csum[:], linb_sb[:])

        wg_sb = const.tile([128, d.KF, H], F32, tag="wg32")
        nc.sync.dma_start(out=wg_sb[:], in_=w_gcn.ap().rearrange("(k p) h -> p k h", p=128))
        wgr = const.tile([128, d.KF, H], BF16, tag="wgr")
        nc.vector.tensor_copy(out=wgr[:], in_=wg_sb[:])

        d1_sb = big.tile([128, 4, d.D1], F32, tag="big")
        nc.sync.dma_start(out=d1_sb[:], in_=dec1_w.ap().rearrange("(b p) d -> p b d", p=128))
        wc_sb = const.tile([128, 3, d.D1], BF16, tag="wcf")
        nc.vector.tensor_add(wc_sb[:, 0, :], d1_sb[:, 0, :], d1_sb[:, 2, :])
        nc.vector.tensor_add(wc_sb[:, 1, :], d1_sb[:, 0, :], d1_sb[:, 3, :])
        nc.vector.tensor_copy(out=wc_sb[:, 2, :], in_=d1_sb[:, 1, :])

        d1b_sb = const.tile([128, MB], F32, tag="d1b")
        nc.sync.dma_start(out=d1b_sb[:], in_=dec1_b.ap().rearrange("(b p) -> p b", p=128))
        d2_sb = const.tile([128, MB, 1], F32, tag="d232")
        nc.sync.dma_start(out=d2_sb[:], in_=dec2_w.ap().rearrange("(b p) o -> p b o", p=128))
        d2r = const.tile([128, MB, 1], BF16, tag="d2r")
        nc.vector.tensor_copy(out=d2r[:], in_=d2_sb[:])
        d2b_sb = const.tile([1, 1], F32, tag="d2b")
        nc.sync.dma_start(out=d2b_sb[:], in_=dec2_b.ap()[:, None])

        # ---------- edge weights (chunk layout) ----------
        def horner(src_ap, n):
            t = big.tile([128, n], F32, tag="big")
            nc.scalar.activation(out=t[:], in_=src_ap, func=AT.Exp, scale=-1.0)
            acc = big.tile([128, n], F32, tag="big")
            nc.vector.tensor_scalar_mul(acc[:], t[:], coef[:, 9:10])
            for k in range(8, -1, -1):
                nc.vector.scalar_tensor_tensor(
                    out=acc[:], in0=acc[:], scalar=coef[:, k:k + 1], in1=t[:],
                    op0=OP.add, op1=OP.mult)
            nc.vector.tensor_scalar_add(acc[:], acc[:], cconst[:])
            return acc

        # ---------- degrees ----------
        WHF = d.WD // d.WH
        WHW = W // d.WH
        deg_r = const.tile([128, W], F32, tag="deg")
        for h in range(d.WH):
            cu = big.tile([128, WHF], F32, tag="big")
            nc.sync.dma_start(out=cu[:], in_=curd.ap()[:, h * WHF:(h + 1) * WHF])
            ewd = horner(cu[:], WHF)
            mk = big.tile([128, WHF], F32, tag="big")
            nc.sync.dma_start(out=mk[:], in_=maskd.ap()[:, h * WHF:(h + 1) * WHF])
            nc.vector.tensor_mul(ewd[:], ewd[:], mk[:])
            nc.vector.tensor_reduce(
                out=deg_r[:, h * WHW:(h + 1) * WHW],
                in_=ewd[:].rearrange("p (w dd) -> p w dd", dd=D),
                axis=mybir.AxisListType.X, op=OP.add)
        nc.vector.tensor_scalar_add(deg_r[:], deg_r[:], 1.0)
        mw = const.tile([128, W], F32, tag="mw")
        nc.vector.tensor_single_scalar(out=mw[:], in_=deg_r[:], scalar=0.0, op=OP.is_gt)
        degm = const.tile([128, W], F32, tag="degm")
        nc.vector.tensor_mul(degm[:], deg_r[:], mw[:])
        onem = const.tile([128, W], F32, tag="onem")
        nc.vector.tensor_scalar(out=onem[:], in0=mw[:], scalar1=-1.0, scalar2=1.0,
                                op0=OP.mult, op1=OP.add)
        nc.vector.tensor_add(degm[:], degm[:], onem[:])
        rec = const.tile([128, W], F32, tag="rec")
        nc.vector.reciprocal(out=rec[:], in_=degm[:])
        dsq = const.tile([128, W], F32, tag="dsq")
        nc.scalar.activation(out=dsq[:], in_=rec[:], func=AT.Sqrt)
        dinv = const.tile([128, W], F32, tag="dinv")
        nc.vector.tensor_mul(dinv[:], dsq[:], mw[:])

        # ---------- xw + y per window ----------
        y_loc = dr.tile([d.NPAD, H], BF16)
        y_loc_r = y_loc[:].rearrange("(w p) h -> p w h", p=128)
        xt_r = xt_d.ap().rearrange("p (k n) -> p k n", k=d.KF)
        GX = 8
        for wb in range((W + GX - 1) // GX):
            w0 = wb * GX
            gw = min(GX, W - w0)
            xc = sb.tile([128, d.KF, GX * 128], BF16, tag="xc", bufs=2)
            nc.sync.dma_start(out=xc[:, :, :gw * 128],
                              in_=xt_r[:, :, w0 * 128:(w0 + gw) * 128])
            for wi in range(gw):
                w = w0 + wi
                pxw = ps.tile([128, H], F32, tag="win")
                for k in range(d.KF):
                    nc.tensor.matmul(pxw[:],
                                     lhsT=xc[:, k, wi * 128:(wi + 1) * 128],
                                     rhs=wgr[:, k, :],
                                     start=(k == 0), stop=(k == d.KF - 1))
                yw = sb.tile([128, H], BF16, tag="yw")
                nc.scalar.activation(out=yw[:], in_=pxw[:], func=AT.Copy,
                                     scale=dinv[:, w:w + 1])
                nc.sync.dma_start(out=y_loc_r[:, w, :], in_=yw[:])

        y_full = dr.tile([NCORES * d.NPAD, H], BF16, addr_space="Shared")
        nc.gpsimd.collective_compute(
            "AllGather", OP.bypass, replica_groups=[list(range(NCORES))],
            ins=[y_loc[:]], outs=[y_full[:]])

        curb_sb = big.tile([128, d.NCH], F32, tag="big")
        nc.sync.dma_start(out=curb_sb[:], in_=curb.ap())
        ew_nch = horner(curb_sb[:], d.NCH)
        ew_f = const.tile([128, d.NCH], BF16, tag="ewf")
        nc.vector.tensor_copy(out=ew_f[:], in_=ew_nch[:])

        dst_sb = big.tile([128, d.NCH], F32, tag="big")
        nc.sync.dma_start(out=dst_sb[:], in_=dstf.ap())
        dst_bf = const.tile([128, d.NCH], BF16, tag="dstbf")
        nc.vector.tensor_copy(out=dst_bf[:], in_=dst_sb[:])

        # ---------- aggregation ----------
        # Window groups of G: one big gather per (segment plane, group), one
        # batched one-hot build + in-place ew fold per (plane, group), then
        # per-window matmul chains accumulating all 4 planes into psum.
        x1_loc = dr.tile([d.NPAD, H], BF16)
        x1_loc_r = x1_loc[:].rearrange("(w p) h -> p w h", p=128)
        G = 4
        for gb in range((W + G - 1) // G):
            w0 = gb * G
            gw = min(G, W - w0)
            yts, sss = [], []
            for s in range(NSEG):
                cs = d.CS[s]
                n_idx = gw * cs * 128
                ix = sb.tile([128, G * cs * 8], I16, tag="ix")
                c0 = d.PB[s] + w0 * cs
                nc.sync.dma_start(out=ix[:, :gw * cs * 8],
                                  in_=g16.ap()[:, c0 * 8:(c0 + gw * cs) * 8])
                yt = gp.tile([128, G * cs, H], BF16, tag="yt", bufs=5)
                nc.gpsimd.dma_gather(
                    out_ap=yt[:, :gw * cs, :],
                    in_ap=y_full[s * d.SEGR:(s + 1) * d.SEGR, :],
                    idxs_ap=ix[:, :gw * cs * 8], num_idxs=n_idx,
                    num_idxs_reg=n_idx, elem_size=H, single_packet=False)
                ss = gp.tile([128, G * cs, 128], BF16, tag="sc", bufs=5)
                nc.vector.tensor_tensor(
                    out=ss[:, :gw * cs, :],
                    in0=dst_bf[:, c0:c0 + gw * cs, None].to_broadcast(
                        [128, gw * cs, 128]),
                    in1=iota_bf[:, None, :].to_broadcast([128, gw * cs, 128]),
                    op=OP.is_equal)
                nc.vector.tensor_tensor(
                    out=ss[:, :gw * cs, :], in0=ss[:, :gw * cs, :],
                    in1=ew_f[:, c0:c0 + gw * cs, None].to_broadcast(
                        [128, gw * cs, 128]),
                    op=OP.mult)
                yts.append(yt)
                sss.append(ss)
            for wi in range(gw):
                w = w0 + wi
                pw = ps.tile([128, H], F32, tag="win")
                first = True
                for s in range(NSEG):
                    cs = d.CS[s]
                    for k in range(cs):
                        j = wi * cs + k
                        last = (s == NSEG - 1) and (k == cs - 1)
                        nc.tensor.matmul(pw[:], lhsT=sss[s][:, j, :],
                                         rhs=yts[s][:, j, :],
                                         start=first, stop=last)
                        first = False
                ywr = sb.tile([128, H], BF16, tag="ywr")
                nc.sync.dma_start(out=ywr[:], in_=y_loc_r[:, w, :])
                t1 = sb.tile([128, H], F32, tag="t1")
                nc.vector.tensor_add(t1[:], pw[:], ywr[:])
                t2 = sb.tile([128, H], F32, tag="t2")
                nc.vector.scalar_tensor_tensor(
                    out=t2[:], in0=t1[:], scalar=dinv[:, w:w + 1], in1=bmat_sb[:],
                    op0=OP.mult, op1=OP.add)
                x1w = sb.tile([128, H], BF16, tag="x1w")
                nc.scalar.activation(out=x1w[:], in_=t2[:], func=AT.Relu)
                nc.sync.dma_start(out=x1_loc_r[:, w, :], in_=x1w[:])

        x1_full = dr.tile([NCORES * d.NPAD, H], BF16, addr_space="Shared")
        nc.gpsimd.collective_compute(
            "AllGather", OP.bypass, replica_groups=[list(range(NCORES))],
            ins=[x1_loc[:]], outs=[x1_full[:]])

        # ---------- pair decoder ----------
        PG = d.PG
        NT2 = 2 * PG // 4               # 512-pair tiles per super-batch
        out_r = out_d.ap().rearrange("a b -> b a")      # [1, PCH2*128]
        for sbch in range(8):            # super-batches of 2 groups
            g0 = sbch * 2
            e1 = gp.tile([128, 2 * PG, H], BF16, tag="et", bufs=5)
            e2 = gp.tile([128, 2 * PG, H], BF16, tag="et", bufs=5)
            for gi in range(2):
                g = g0 + gi
                a_seg, b_seg = g // NSEG, g % NSEG
                for (tile_, src_seg, arr) in ((e1, a_seg, o1g), (e2, b_seg, o2g)):
                    c0 = g * PG
                    ix = sb.tile([128, PG * 8], I16, tag="ixd")
                    nc.sync.dma_start(out=ix[:], in_=arr.ap()[:, c0 * 8:(c0 + PG) * 8])
                    nc.gpsimd.dma_gather(
                        out_ap=tile_[:, gi * PG:(gi + 1) * PG, :],
                        in_ap=x1_full[src_seg * d.SEGR:(src_seg + 1) * d.SEGR, :],
                        idxs_ap=ix[:], num_idxs=PG * 128, num_idxs_reg=PG * 128,
                        elem_size=H, single_packet=False)
            em = gp.tile([128, 2 * PG, H], BF16, tag="et", bufs=5)
            nc.vector.tensor_mul(em[:], e1[:], e2[:])
            ob = sb.tile([1, NT2 * 512], F32, tag="ob")
            for nt in range(NT2):
                cT = sb.tile([128, 3, 4, 128], BF16, tag="cT")
                for jj in range(4):
                    j = nt * 4 + jj
                    for cix, srct in enumerate((e1, e2, em)):
                        pt = ps.tile([128, 128], BF16, tag="trf")
                        nc.tensor.transpose(pt[:], srct[:, j, :], ident[:])
                        if (j + cix) % 2 == 0:
                            nc.scalar.copy(out=cT[:, cix, jj, :], in_=pt[:])
                        else:
                            nc.vector.tensor_copy(out=cT[:, cix, jj, :], in_=pt[:])
                h_sb = sb.tile([128, MB, 512], BF16, tag="hsb")
                for mb in range(MB):
                    ph = ps.tile([128, 512], F32, tag="ph")
                    for cix in range(3):
                        nc.tensor.matmul(
                            ph[:], lhsT=wc_sb[:, cix, mb * 128:(mb + 1) * 128],
                            rhs=cT[:, cix, :, :],
                            start=(cix == 0), stop=(cix == 2))
                    nc.scalar.activation(out=h_sb[:, mb, :], in_=ph[:], func=AT.Relu,
                                         bias=d1b_sb[:, mb:mb + 1])
                po = ps.tile([1, 512], F32, tag="po")
                for mb in range(MB):
                    nc.tensor.matmul(po[:], lhsT=d2r[:, mb, :],
                                     rhs=h_sb[:, mb, :],
                                     start=(mb == 0), stop=(mb == MB - 1))
                nc.scalar.activation(out=ob[:, nt * 512:(nt + 1) * 512], in_=po[:],
                                     func=AT.Identity, bias=d2b_sb[:, :])
            off = sbch * NT2 * 512
            nc.sync.dma_start(out=out_r[:, off:off + NT2 * 512], in_=ob[:])

    nc.compile()
    return nc


_CACHE = {}
TRACE = False          # test harness sets True to capture NTFF profile
LAST_RESULT = None     # BassKernelResults of the most recent run


def kernel(**inputs):
    x = np.asarray(inputs["x"], np.float32)
    curvature = np.asarray(inputs["curvature"], np.float32)
    edge_index = np.asarray(inputs["edge_index"])
    idx = np.asarray(inputs["idx"])
    N, FIN = x.shape
    E = edge_index.shape[1]
    P = idx.shape[1]
    H = np.asarray(inputs["W_gcn"]).shape[1]
    D1 = np.asarray(inputs["dec1_w"]).shape[1]

    CS, D, PG = chunk_params(edge_index, idx, N)
    d = Dims(N, E, P, FIN, H, D1, CS, D, PG)
    maps, perms = preprocess(d, x, curvature, edge_index, idx,
                             inputs["b_gcn"], inputs["lin1_w"], inputs["lin1_b"])
    shared = dict(
        w_gcn=np.asarray(inputs["W_gcn"], np.float32),
        dec1_w=np.asarray(inputs["dec1_w"], np.float32),
        dec1_b=np.asarray(inputs["dec1_b"], np.float32).reshape(-1),
        dec2_w=np.asarray(inputs["dec2_w"], np.float32),
        dec2_b=np.asarray(inputs["dec2_b"], np.float32).reshape(-1),
    )
    for m in maps:
        m.update(shared)

    key = (N, E, P, tuple(CS), D, PG)
    if key not in _CACHE:
        _CACHE[key] = build(d)
    nc = _CACHE[key]

    from concourse.bass_utils import run_bass_kernel_spmd
    res = run_bass_kernel_spmd(nc, maps, core_ids=list(range(NCORES)),
                               trace=TRACE)
    global LAST_RESULT
    LAST_RESULT = res
    out = np.empty((P, 1), np.float32)
    for c in range(NCORES):
        vals = np.asarray(res.results[c]["out_d"], np.float32)[:, 0]
        pk, slot = perms[c]
        out[c * d.PPC + pk, 0] = vals[slot]
    return out



# revision 40
# speedup vs baseline: 1.1554x; 1.1554x over previous
"""Trainium2 Bass kernel for nn_CGCN (curvature-weighted GCN + pair decoder).

Strategy (8 NeuronCores, SPMD):
  - Edges sharded by DESTINATION node: core c owns nodes [c*N/8, (c+1)*N/8)
    and every edge whose col lands there (host bins/sorts; index plumbing
    only). The scatter-add stays core-local - no [N,H] all-reduce.
  - Edge weights ew = Linear(func_k(curvature)) on device via Horner in
    t = exp(-c).
  - Degrees: per-node padded layout [node-on-partition, window x Dmax],
    fp32 vector reduce (exact per-node sums, no long-prefix cancellation).
  - y = dinv * (x @ W) per 128-node window (bf16), AllGathered.
  - Aggregation: per 128-edge chunk, S[p,j] = ew[p] * (dst_rel[p]==j) built
    on DVE; psum[window,H] += S^T @ y_rows on PE. y rows fetched with
    dma_gather (int16 idx), the table split in 4 segments of 2*NPAD rows so
    indices fit int16; chunks are organized in 4 segment planes.
  - x1 = relu(dinv*(agg+y)+b), AllGathered (bf16).
  - Decoder: pairs grouped by (seg(e1), seg(e2)) into 16 padded groups so
    each dma_gather call has a single segment base; host un-permutes the
    output. feat@dec1_w decomposed as e1@(Wa+Wc) + e2@(Wa+Wd) + (e1*e2)@Wb.
"""
import sys

for _p in ("/opt/trn_rl_repo",):
    if _p not in sys.path:
        sys.path.append(_p)

import numpy as np
from contextlib import ExitStack

import concourse.bass as bass
import concourse.tile as tile
from concourse import mybir, bacc
from concourse.masks import make_identity

F32 = mybir.dt.float32
F32R = mybir.dt.float32r
BF16 = mybir.dt.bfloat16
I32 = mybir.dt.int32
I16 = mybir.dt.int16

NCORES = 8
NSEG = 4          # table segments (2 cores' rows each) so idx fits int16


def _wrap16(a):
    """[128, n] per-slot idx array -> dma_gather idx layout [128, n*8] int16.

    Slot (chunk c, partition p) maps to idx-list position i = c*128 + p;
    the ucode reads position i from [p16=i%16, col=c*8 + (i%128)//16],
    partitions replicated x8.
    """
    p128, n = a.shape
    assert p128 == 128
    m = np.zeros((16, n, 8), np.int16)
    for s16 in range(8):
        m[:, :, s16] = a[s16 * 16:(s16 + 1) * 16, :]
    m = m.reshape(16, n * 8)
    return np.tile(m, (8, 1)).copy()


class Dims:
    def __init__(self, N, E, P, FIN, H, D1, CS, D, PG):
        self.N, self.E, self.P = N, E, P
        self.FIN, self.H, self.D1 = FIN, H, D1
        self.CS = list(CS)             # chunks per window per segment plane
        self.D = D                     # max node in-degree
        self.PG = PG                   # decoder chunks per pair-group (even)
        self.NSH = N // NCORES
        self.W = (self.NSH + 127) // 128
        self.NPAD = self.W * 128
        self.SEGR = 2 * self.NPAD      # rows per table segment
        self.NCH = self.W * sum(CS)    # total chunk columns
        self.PB = [self.W * sum(CS[:s]) for s in range(NSEG)]  # plane col base
        self.WD = self.W * D
        self.PPC = P // NCORES
        self.PCH = self.PPC // 128     # real decoder chunks
        self.PCH2 = 16 * PG            # padded decoder chunks (16 groups)
        self.KF = FIN // 128
        self.GBW = 1   # >1 risks SWDGE ring/packet overflow (561-desc calls hang HW)
        self.WH = 2 if self.W % 2 == 0 else 1
        assert self.SEGR <= 32768
        assert PG % 2 == 0


def chunk_params(edge_index, idx, N):
    """Uniform (CS per segment, D, PG) from the data."""
    NSH = N // NCORES
    W = (NSH + 127) // 128
    NPAD = W * 128
    col = np.asarray(edge_index[1], np.int64)
    row = np.asarray(edge_index[0], np.int64)
    order = np.argsort(col, kind="stable")
    col_s, row_s = col[order], row[order]
    D = int(np.bincount(col_s, minlength=N).max())
    seg_of_row = ((row_s // NSH) * NPAD + (row_s % NSH)) // (2 * NPAD)
    CS = [1] * NSEG
    for c in range(NCORES):
        lo, hi = np.searchsorted(col_s, [c * NSH, (c + 1) * NSH])
        wof = (col_s[lo:hi] - c * NSH) >> 7
        sof = seg_of_row[lo:hi]
        cnt = np.bincount(wof * NSEG + sof, minlength=W * NSEG).reshape(W, NSEG)
        for s in range(NSEG):
            CS[s] = max(CS[s], int(np.ceil(cnt[:, s].max() / 128)))
    # decoder groups
    i1 = np.asarray(idx[0], np.int64)
    i2 = np.asarray(idx[1], np.int64)
    g1 = ((i1 // NSH) * NPAD + (i1 % NSH)) // (2 * NPAD)
    g2 = ((i2 // NSH) * NPAD + (i2 % NSH)) // (2 * NPAD)
    PPC = i1.shape[0] // NCORES
    PG = 1
    for c in range(NCORES):
        g = g1[c * PPC:(c + 1) * PPC] * NSEG + g2[c * PPC:(c + 1) * PPC]
        cnt = np.bincount(g, minlength=NSEG * NSEG)
        PG = max(PG, int(np.ceil(cnt.max() / 128)))
    if PG % 2:
        PG += 1
    return CS, D, PG


def preprocess(d, x, curvature, edge_index, idx, b_gcn, lin1_w, lin1_b):
    """Index plumbing: sort/bin/pad edges and pairs, build per-core inputs."""
    row = np.asarray(edge_index[0], dtype=np.int64)
    col = np.asarray(edge_index[1], dtype=np.int64)
    cur = np.asarray(curvature, dtype=np.float32)
    order = np.argsort(col, kind="stable")
    row_s, col_s, cur_s = row[order], col[order], cur[order]
    bounds = np.searchsorted(col_s, np.arange(NCORES + 1) * d.NSH)

    iota_f = np.broadcast_to(np.arange(128, dtype=np.float32), (128, 128)).copy()
    b_mat = np.broadcast_to(np.asarray(b_gcn, np.float32)[None, :], (128, d.H)).copy()
    linw_rep = np.broadcast_to(np.asarray(lin1_w, np.float32)[:, 0][None, :], (128, 10)).copy()
    linb_rep = np.full((128, 1), np.float32(np.asarray(lin1_b).reshape(-1)[0]), np.float32)

    def gidx(nodes):
        return ((nodes // d.NSH) * d.NPAD + (nodes % d.NSH)).astype(np.int64)

    maps = []
    perms = []
    for c in range(NCORES):
        lo, hi = bounds[c], bounds[c + 1]
        colr = (col_s[lo:hi] - c * d.NSH).astype(np.int64)
        grow = gidx(row_s[lo:hi])
        curc = cur_s[lo:hi]
        n_c = colr.shape[0]
        wof = colr >> 7
        sof = grow // d.SEGR
        # order edges by (window, segment) [stable within = by col]
        okey = np.argsort(wof * NSEG + sof, kind="stable")
        colr, grow, curc = colr[okey], grow[okey], curc[okey]
        wof, sof = wof[okey], sof[okey]
        # chunk column for each edge
        cnt = np.bincount(wof * NSEG + sof, minlength=d.W * NSEG).reshape(d.W, NSEG)
        start = np.zeros(d.W * NSEG, np.int64)
        start[1:] = np.cumsum(cnt.reshape(-1))[:-1]
        pos = np.arange(n_c) - start[wof * NSEG + sof]
        ci = np.array(d.PB, np.int64)[sof] + wof * np.array(d.CS, np.int64)[sof] + (pos >> 7)
        p = (pos & 127).astype(np.int64)

        dstf = np.full((128, d.NCH), -1.0, np.float32)
        dstf[p, ci] = (colr & 127).astype(np.float32)
        curb = np.zeros((128, d.NCH), np.float32)
        curb[p, ci] = curc
        ridx = np.zeros((128, d.NCH), np.int64)
        ridx[p, ci] = grow - sof * d.SEGR
        g16 = _wrap16(ridx.astype(np.int16))

        # degree layout (per-node padded)
        colr2 = col_s[lo:hi] - c * d.NSH
        cur2 = cur_s[lo:hi]
        nstart = np.searchsorted(colr2, np.arange(d.NSH))
        dpos = np.arange(n_c) - nstart[colr2]
        pn = (colr2 & 127).astype(np.int64)
        fi = (colr2 >> 7) * d.D + dpos
        curd = np.zeros((128, d.WD), np.float32)
        curd[pn, fi] = cur2
        maskd = np.zeros((128, d.WD), np.float32)
        maskd[pn, fi] = 1.0

        # host-transposed x shard, bf16: xt[p, k*NPAD + n] = x[base+n, k*128+p]
        x_pad = np.zeros((d.NPAD, d.FIN), np.float32)
        x_pad[:d.NSH] = x[c * d.NSH:(c + 1) * d.NSH]
        import ml_dtypes
        xt = np.ascontiguousarray(
            x_pad.reshape(d.NPAD, d.KF, 128).transpose(2, 1, 0)
        ).reshape(128, d.KF * d.NPAD).astype(ml_dtypes.bfloat16)

        # decoder pair groups
        gi1 = gidx(np.asarray(idx[0][c * d.PPC:(c + 1) * d.PPC], np.int64))
        gi2 = gidx(np.asarray(idx[1][c * d.PPC:(c + 1) * d.PPC], np.int64))
        grp = (gi1 // d.SEGR) * NSEG + (gi2 // d.SEGR)
        pkey = np.argsort(grp, kind="stable")
        gcnt = np.bincount(grp, minlength=16)
        gstart = np.zeros(16, np.int64)
        gstart[1:] = np.cumsum(gcnt)[:-1]
        ppos = np.arange(d.PPC) - gstart[grp[pkey]]
        slot = grp[pkey] * d.PG * 128 + ppos        # slot in padded layout
        o1a = np.zeros((128, d.PCH2), np.int64)
        o2a = np.zeros((128, d.PCH2), np.int64)
        o1a[slot & 127, slot >> 7] = gi1[pkey] % d.SEGR
        o2a[slot & 127, slot >> 7] = gi2[pkey] % d.SEGR
        o1g = _wrap16(o1a.astype(np.int16))
        o2g = _wrap16(o2a.astype(np.int16))
        perms.append(np.asarray(slot))              # real pair pkey[j] -> slot[j]
        pk = np.empty(d.PPC, np.int64)
        pk[:] = pkey
        perms[-1] = (pk, np.asarray(slot))

        maps.append(dict(
            xt=xt, curb=curb, dstf=dstf, g16=g16,
            curd=curd, maskd=maskd, o1g=o1g, o2g=o2g,
            iota_f=iota_f, b_mat=b_mat, linw=linw_rep, linb=linb_rep,
        ))
    return maps, perms


def build(d):
    nc = bacc.Bacc("TRN2", target_bir_lowering=False, debug=False,
                   num_devices=NCORES)
    H, D, W = d.H, d.D, d.W

    xt_d = nc.dram_tensor("xt", [128, d.KF * d.NPAD], BF16, kind="ExternalInput")
    curb = nc.dram_tensor("curb", [128, d.NCH], F32, kind="ExternalInput")
    dstf = nc.dram_tensor("dstf", [128, d.NCH], F32, kind="ExternalInput")
    g16 = nc.dram_tensor("g16", [128, d.NCH * 8], I16, kind="ExternalInput")
    curd = nc.dram_tensor("curd", [128, d.WD], F32, kind="ExternalInput")
    maskd = nc.dram_tensor("maskd", [128, d.WD], F32, kind="ExternalInput")
    o1g = nc.dram_tensor("o1g", [128, d.PCH2 * 8], I16, kind="ExternalInput")
    o2g = nc.dram_tensor("o2g", [128, d.PCH2 * 8], I16, kind="ExternalInput")
    iota_f = nc.dram_tensor("iota_f", [128, 128], F32, kind="ExternalInput")
    b_mat = nc.dram_tensor("b_mat", [128, H], F32, kind="ExternalInput")
    linw = nc.dram_tensor("linw", [128, 10], F32, kind="ExternalInput")
    linb = nc.dram_tensor("linb", [128, 1], F32, kind="ExternalInput")
    w_gcn = nc.dram_tensor("w_gcn", [d.FIN, H], F32, kind="ExternalInput")
    dec1_w = nc.dram_tensor("dec1_w", [4 * H, d.D1], F32, kind="ExternalInput")
    dec1_b = nc.dram_tensor("dec1_b", [d.D1], F32, kind="ExternalInput")
    dec2_w = nc.dram_tensor("dec2_w", [d.D1, 1], F32, kind="ExternalInput")
    dec2_b = nc.dram_tensor("dec2_b", [1], F32, kind="ExternalInput")
    out_d = nc.dram_tensor("out_d", [d.PCH2 * 128, 1], F32, kind="ExternalOutput")

    AT = mybir.ActivationFunctionType
    OP = mybir.AluOpType
    MB = d.D1 // 128

    with ExitStack() as ctx:
        tc = ctx.enter_context(tile.TileContext(nc))
        const = ctx.enter_context(tc.tile_pool(name="const", bufs=1))
        sb = ctx.enter_context(tc.tile_pool(name="sb", bufs=3))
        big = ctx.enter_context(tc.tile_pool(name="big", bufs=3))
        gp = ctx.enter_context(tc.tile_pool(name="gp", bufs=5))   # gather tiles
        ps = ctx.enter_context(tc.tile_pool(name="ps", bufs=2, space="PSUM"))
        dr = ctx.enter_context(tc.tile_pool(name="dr", bufs=1, space="DRAM"))

        # ---------- constants ----------
        iota_sb = const.tile([128, 128], F32, tag="iota32")
        nc.sync.dma_start(out=iota_sb[:], in_=iota_f.ap())
        iota_bf = const.tile([128, 128], BF16, tag="iotabf")
        nc.vector.tensor_copy(out=iota_bf[:], in_=iota_sb[:])
        ident = const.tile([128, 128], BF16, tag="ident")
        make_identity(nc, ident[:])
        ident_f = const.tile([128, 128], F32, tag="identf")
        make_identity(nc, ident_f[:])
        bmat_sb = const.tile([128, H], F32, tag="bmat")
        nc.sync.dma_start(out=bmat_sb[:], in_=b_mat.ap())

        linw_sb = const.tile([128, 10], F32, tag="linw")
        nc.sync.dma_start(out=linw_sb[:], in_=linw.ap())
        linb_sb = const.tile([128, 1], F32, tag="linb")
        nc.sync.dma_start(out=linb_sb[:], in_=linb.ap())
        coef = const.tile([128, 10], F32, tag="coef")
        nc.vector.tensor_scalar_mul(coef[:], linw_sb[:], 0.5)
        csum = const.tile([128, 1], F32, tag="csum")
        nc.vector.tensor_reduce(out=csum[:], in_=coef[:], axis=mybir.AxisListType.X,
                                op=OP.add)
        cconst = const.tile([128, 1], F32, tag="cconst")
        nc.vector.tensor_add(cconst[:], csum[:], linb_sb[:])

        wg_sb = const.tile([128, d.KF, H], F32, tag="wg32")
        nc.sync.dma_start(out=wg_sb[:], in_=w_gcn.ap().rearrange("(k p) h -> p k h", p=128))
        wgr = const.tile([128, d.KF, H], BF16, tag="wgr")
        nc.vector.tensor_copy(out=wgr[:], in_=wg_sb[:])

        d1_sb = big.tile([128, 4, d.D1], F32, tag="big")
        nc.sync.dma_start(out=d1_sb[:], in_=dec1_w.ap().rearrange("(b p) d -> p b d", p=128))
        wc_sb = const.tile([128, 3, d.D1], BF16, tag="wcf")
        nc.vector.tensor_add(wc_sb[:, 0, :], d1_sb[:, 0, :], d1_sb[:, 2, :])
        nc.vector.tensor_add(wc_sb[:, 1, :], d1_sb[:, 0, :], d1_sb[:, 3, :])
        nc.vector.tensor_copy(out=wc_sb[:, 2, :], in_=d1_sb[:, 1, :])

        d1b_sb = const.tile([128, MB], F32, tag="d1b")
        nc.sync.dma_start(out=d1b_sb[:], in_=dec1_b.ap().rearrange("(b p) -> p b", p=128))
        d2_sb = const.tile([128, MB, 1], F32, tag="d232")
        nc.sync.dma_start(out=d2_sb[:], in_=dec2_w.ap().rearrange("(b p) o -> p b o", p=128))
        d2r = const.tile([128, MB, 1], BF16, tag="d2r")
        nc.vector.tensor_copy(out=d2r[:], in_=d2_sb[:])
        d2b_sb = const.tile([1, 1], F32, tag="d2b")
        nc.sync.dma_start(out=d2b_sb[:], in_=dec2_b.ap()[:, None])

        # ---------- edge weights (chunk layout) ----------
        def horner(src_ap, n):
            t = big.tile([128, n], F32, tag="big")
            nc.scalar.activation(out=t[:], in_=src_ap, func=AT.Exp, scale=-1.0)
            acc = big.tile([128, n], F32, tag="big")
            nc.vector.tensor_scalar_mul(acc[:], t[:], coef[:, 9:10])
            for k in range(8, -1, -1):
                nc.vector.scalar_tensor_tensor(
                    out=acc[:], in0=acc[:], scalar=coef[:, k:k + 1], in1=t[:],
                    op0=OP.add, op1=OP.mult)
            nc.vector.tensor_scalar_add(acc[:], acc[:], cconst[:])
            return acc

        # ---------- degrees ----------
        WHF = d.WD // d.WH
        WHW = W // d.WH
        deg_r = const.tile([128, W], F32, tag="deg")
        for h in range(d.WH):
            cu = big.tile([128, WHF], F32, tag="big")
            nc.sync.dma_start(out=cu[:], in_=curd.ap()[:, h * WHF:(h + 1) * WHF])
            ewd = horner(cu[:], WHF)
            mk = big.tile([128, WHF], F32, tag="big")
            nc.sync.dma_start(out=mk[:], in_=maskd.ap()[:, h * WHF:(h + 1) * WHF])
            nc.vector.tensor_mul(ewd[:], ewd[:], mk[:])
            nc.vector.tensor_reduce(
                out=deg_r[:, h * WHW:(h + 1) * WHW],
                in_=ewd[:].rearrange("p (w dd) -> p w dd", dd=D),
                axis=mybir.AxisListType.X, op=OP.add)
        nc.vector.tensor_scalar_add(deg_r[:], deg_r[:], 1.0)
        mw = const.tile([128, W], F32, tag="mw")
        nc.vector.tensor_single_scalar(out=mw[:], in_=deg_r[:], scalar=0.0, op=OP.is_gt)
        degm = const.tile([128, W], F32, tag="degm")
        nc.vector.tensor_mul(degm[:], deg_r[:], mw[:])
        onem = const.tile([128, W], F32, tag="onem")
        nc.vector.tensor_scalar(out=onem[:], in0=mw[:], scalar1=-1.0, scalar2=1.0,
                                op0=OP.mult, op1=OP.add)
        nc.vector.tensor_add(degm[:], degm[:], onem[:])
        rec = const.tile([128, W], F32, tag="rec")
        nc.vector.reciprocal(out=rec[:], in_=degm[:])
        dsq = const.tile([128, W], F32, tag="dsq")
        nc.scalar.activation(out=dsq[:], in_=rec[:], func=AT.Sqrt)
        dinv = const.tile([128, W], F32, tag="dinv")
        nc.vector.tensor_mul(dinv[:], dsq[:], mw[:])

        # ---------- xw + y per window ----------
        y_loc = dr.tile([d.NPAD, H], BF16)
        y_loc_r = y_loc[:].rearrange("(w p) h -> p w h", p=128)
        xt_r = xt_d.ap().rearrange("p (k n) -> p k n", k=d.KF)
        GX = 4
        for wb in range((W + GX - 1) // GX):
            w0 = wb * GX
            gw = min(GX, W - w0)
            xc = sb.tile([128, d.KF, GX * 128], BF16, tag="xc", bufs=2)
            nc.sync.dma_start(out=xc[:, :, :gw * 128],
                              in_=xt_r[:, :, w0 * 128:(w0 + gw) * 128])
            for wi in range(gw):
                w = w0 + wi
                pxw = ps.tile([128, H], F32, tag="win")
                for k in range(d.KF):
                    nc.tensor.matmul(pxw[:],
                                     lhsT=xc[:, k, wi * 128:(wi + 1) * 128],
                                     rhs=wgr[:, k, :],
                                     start=(k == 0), stop=(k == d.KF - 1))
                yw = sb.tile([128, H], BF16, tag="yw")
                nc.scalar.activation(out=yw[:], in_=pxw[:], func=AT.Copy,
                                     scale=dinv[:, w:w + 1])
                nc.sync.dma_start(out=y_loc_r[:, w, :], in_=yw[:])

        y_full = dr.tile([NCORES * d.NPAD, H], BF16, addr_space="Shared")
        nc.gpsimd.collective_compute(
            "AllGather", OP.bypass, replica_groups=[list(range(NCORES))],
            ins=[y_loc[:]], outs=[y_full[:]])

        curb_sb = big.tile([128, d.NCH], F32, tag="big")
        nc.sync.dma_start(out=curb_sb[:], in_=curb.ap())
        ew_nch = horner(curb_sb[:], d.NCH)
        ew_f = const.tile([128, d.NCH], BF16, tag="ewf")
        nc.vector.tensor_copy(out=ew_f[:], in_=ew_nch[:])

        dst_sb = big.tile([128, d.NCH], F32, tag="big")
        nc.sync.dma_start(out=dst_sb[:], in_=dstf.ap())
        dst_bf = const.tile([128, d.NCH], BF16, tag="dstbf")
        nc.vector.tensor_copy(out=dst_bf[:], in_=dst_sb[:])

        # ---------- aggregation ----------
        # Window groups of G: one big gather per (segment plane, group), one
        # batched one-hot build + in-place ew fold per (plane, group), then
        # per-window matmul chains accumulating all 4 planes into psum.
        x1_loc = dr.tile([d.NPAD, H], BF16)
        x1_loc_r = x1_loc[:].rearrange("(w p) h -> p w h", p=128)
        G = 4
        for gb in range((W + G - 1) // G):
            w0 = gb * G
            gw = min(G, W - w0)
            yts, sss = [], []
            for s in range(NSEG):
                cs = d.CS[s]
                n_idx = gw * cs * 128
                ix = sb.tile([128, G * cs * 8], I16, tag="ix")
                c0 = d.PB[s] + w0 * cs
                nc.sync.dma_start(out=ix[:, :gw * cs * 8],
                                  in_=g16.ap()[:, c0 * 8:(c0 + gw * cs) * 8])
                yt = gp.tile([128, G * cs, H], BF16, tag="yt", bufs=5)
                nc.gpsimd.dma_gather(
                    out_ap=yt[:, :gw * cs, :],
                    in_ap=y_full[s * d.SEGR:(s + 1) * d.SEGR, :],
                    idxs_ap=ix[:, :gw * cs * 8], num_idxs=n_idx,
                    num_idxs_reg=n_idx, elem_size=H, single_packet=False)
                ss = gp.tile([128, G * cs, 128], BF16, tag="sc", bufs=5)
                nc.vector.tensor_tensor(
                    out=ss[:, :gw * cs, :],
                    in0=dst_bf[:, c0:c0 + gw * cs, None].to_broadcast(
                        [128, gw * cs, 128]),
                    in1=iota_bf[:, None, :].to_broadcast([128, gw * cs, 128]),
                    op=OP.is_equal)
                nc.vector.tensor_tensor(
                    out=ss[:, :gw * cs, :], in0=ss[:, :gw * cs, :],
                    in1=ew_f[:, c0:c0 + gw * cs, None].to_broadcast(
                        [128, gw * cs, 128]),
                    op=OP.mult)
                yts.append(yt)
                sss.append(ss)
            for wi in range(gw):
                w = w0 + wi
                pw = ps.tile([128, H], F32, tag="win")
                first = True
                for s in range(NSEG):
                    cs = d.CS[s]
                    for k in range(cs):
                        j = wi * cs + k
                        last = (s == NSEG - 1) and (k == cs - 1)
                        nc.tensor.matmul(pw[:], lhsT=sss[s][:, j, :],
                                         rhs=yts[s][:, j, :],
                                         start=first, stop=last)
                        first = False
                ywr = sb.tile([128, H], BF16, tag="ywr")
                nc.sync.dma_start(out=ywr[:], in_=y_loc_r[:, w, :])
                t1 = sb.tile([128, H], F32, tag="t1")
                nc.vector.tensor_add(t1[:], pw[:], ywr[:])
                t2 = sb.tile([128, H], F32, tag="t2")
                nc.vector.scalar_tensor_tensor(
                    out=t2[:], in0=t1[:], scalar=dinv[:, w:w + 1], in1=bmat_sb[:],
                    op0=OP.mult, op1=OP.add)
                x1w = sb.tile([128, H], BF16, tag="x1w")
                nc.scalar.activation(out=x1w[:], in_=t2[:], func=AT.Relu)
                nc.sync.dma_start(out=x1_loc_r[:, w, :], in_=x1w[:])

        x1_full = dr.tile([NCORES * d.NPAD, H], BF16, addr_space="Shared")
        nc.gpsimd.collective_compute(
            "AllGather", OP.bypass, replica_groups=[list(range(NCORES))],
            ins=[x1_loc[:]], outs=[x1_full[:]])

        # ---------- pair decoder ----------
        PG = d.PG
        NT2 = 2 * PG // 4               # 512-pair tiles per super-batch
        out_r = out_d.ap().rearrange("a b -> b a")      # [1, PCH2*128]
        for sbch in range(8):            # super-batches of 2 groups
            g0 = sbch * 2
            e1 = gp.tile([128, 2 * PG, H], BF16, tag="et", bufs=4)
            e2 = gp.tile([128, 2 * PG, H], BF16, tag="et", bufs=4)
            for gi in range(2):
                g = g0 + gi
                a_seg, b_seg = g // NSEG, g % NSEG
                for (tile_, src_seg, arr) in ((e1, a_seg, o1g), (e2, b_seg, o2g)):
                    c0 = g * PG
                    ix = sb.tile([128, PG * 8], I16, tag="ixd")
                    nc.sync.dma_start(out=ix[:], in_=arr.ap()[:, c0 * 8:(c0 + PG) * 8])
                    nc.gpsimd.dma_gather(
                        out_ap=tile_[:, gi * PG:(gi + 1) * PG, :],
                        in_ap=x1_full[src_seg * d.SEGR:(src_seg + 1) * d.SEGR, :],
                        idxs_ap=ix[:], num_idxs=PG * 128, num_idxs_reg=PG * 128,
                        elem_size=H, single_packet=False)
            em = gp.tile([128, 2 * PG, H], BF16, tag="et", bufs=4)
            nc.vector.tensor_mul(em[:], e1[:], e2[:])
            ob = sb.tile([1, NT2 * 512], F32, tag="ob")
            for nt in range(NT2):
                cT = sb.tile([128, 3, 4, 128], BF16, tag="cT")
                for jj in range(4):
                    j = nt * 4 + jj
                    for cix, srct in enumerate((e1, e2, em)):
                        pt = ps.tile([128, 128], BF16, tag="trf")
                        nc.tensor.transpose(pt[:], srct[:, j, :], ident[:])
                        if (j + cix) % 2 == 0:
                            nc.scalar.copy(out=cT[:, cix, jj, :], in_=pt[:])
                        else:
                            nc.vector.tensor_copy(out=cT[:, cix, jj, :], in_=pt[:])
                h_sb = sb.tile([128, MB, 512], BF16, tag="hsb")
                for mb in range(MB):
                    ph = ps.tile([128, 512], F32, tag="ph")
                    for cix in range(3):
                        nc.tensor.matmul(
                            ph[:], lhsT=wc_sb[:, cix, mb * 128:(mb + 1) * 128],
                            rhs=cT[:, cix, :, :],
                            start=(cix == 0), stop=(cix == 2))
                    nc.scalar.activation(out=h_sb[:, mb, :], in_=ph[:], func=AT.Relu,
                                         bias=d1b_sb[:, mb:mb + 1])
                po = ps.tile([1, 512], F32, tag="po")
                for mb in range(MB):
                    nc.tensor.matmul(po[:], lhsT=d2r[:, mb, :],
                                     rhs=h_sb[:, mb, :],
                                     start=(mb == 0), stop=(mb == MB - 1))
                nc.scalar.activation(out=ob[:, nt * 512:(nt + 1) * 512], in_=po[:],
                                     func=AT.Identity, bias=d2b_sb[:, :])
            off = sbch * NT2 * 512
            nc.sync.dma_start(out=out_r[:, off:off + NT2 * 512], in_=ob[:])

    nc.compile()
    return nc


_CACHE = {}
TRACE = False          # test harness sets True to capture NTFF profile
LAST_RESULT = None     # BassKernelResults of the most recent run


def kernel(**inputs):
    x = np.asarray(inputs["x"], np.float32)
    curvature = np.asarray(inputs["curvature"], np.float32)
    edge_index = np.asarray(inputs["edge_index"])
    idx = np.asarray(inputs["idx"])
    N, FIN = x.shape
    E = edge_index.shape[1]
    P = idx.shape[1]
    H = np.asarray(inputs["W_gcn"]).shape[1]
    D1 = np.asarray(inputs["dec1_w"]).shape[1]

    CS, D, PG = chunk_params(edge_index, idx, N)
    d = Dims(N, E, P, FIN, H, D1, CS, D, PG)
    maps, perms = preprocess(d, x, curvature, edge_index, idx,
                             inputs["b_gcn"], inputs["lin1_w"], inputs["lin1_b"])
    shared = dict(
        w_gcn=np.asarray(inputs["W_gcn"], np.float32),
        dec1_w=np.asarray(inputs["dec1_w"], np.float32),
        dec1_b=np.asarray(inputs["dec1_b"], np.float32).reshape(-1),
        dec2_w=np.asarray(inputs["dec2_w"], np.float32),
        dec2_b=np.asarray(inputs["dec2_b"], np.float32).reshape(-1),
    )
    for m in maps:
        m.update(shared)

    key = (N, E, P, tuple(CS), D, PG)
    if key not in _CACHE:
        _CACHE[key] = build(d)
    nc = _CACHE[key]

    from concourse.bass_utils import run_bass_kernel_spmd
    res = run_bass_kernel_spmd(nc, maps, core_ids=list(range(NCORES)),
                               trace=TRACE)
    global LAST_RESULT
    LAST_RESULT = res
    out = np.empty((P, 1), np.float32)
    for c in range(NCORES):
        vals = np.asarray(res.results[c]["out_d"], np.float32)[:, 0]
        pk, slot = perms[c]
        out[c * d.PPC + pk, 0] = vals[slot]
    return out



# revision 41
# speedup vs baseline: 1.2421x; 1.0750x over previous
"""Trainium2 Bass kernel for nn_CGCN (curvature-weighted GCN + pair decoder).

Strategy (8 NeuronCores, SPMD):
  - Edges sharded by DESTINATION node: core c owns nodes [c*N/8, (c+1)*N/8)
    and every edge whose col lands there (host bins/sorts; index plumbing
    only). The scatter-add stays core-local - no [N,H] all-reduce.
  - Edge weights ew = Linear(func_k(curvature)) on device via Horner in
    t = exp(-c).
  - Degrees: per-node padded layout [node-on-partition, window x Dmax],
    fp32 vector reduce (exact per-node sums, no long-prefix cancellation).
  - y = dinv * (x @ W) per 128-node window (bf16), AllGathered.
  - Aggregation: per 128-edge chunk, S[p,j] = ew[p] * (dst_rel[p]==j) built
    on DVE; psum[window,H] += S^T @ y_rows on PE. y rows fetched with
    dma_gather (int16 idx), the table split in 4 segments of 2*NPAD rows so
    indices fit int16; chunks are organized in 4 segment planes.
  - x1 = relu(dinv*(agg+y)+b), AllGathered (bf16).
  - Decoder: pairs grouped by (seg(e1), seg(e2)) into 16 padded groups so
    each dma_gather call has a single segment base; host un-permutes the
    output. feat@dec1_w decomposed as e1@(Wa+Wc) + e2@(Wa+Wd) + (e1*e2)@Wb.
"""
import sys

for _p in ("/opt/trn_rl_repo",):
    if _p not in sys.path:
        sys.path.append(_p)

import numpy as np
from contextlib import ExitStack

import concourse.bass as bass
import concourse.tile as tile
from concourse import mybir, bacc
from concourse.masks import make_identity

F32 = mybir.dt.float32
F32R = mybir.dt.float32r
BF16 = mybir.dt.bfloat16
I32 = mybir.dt.int32
I16 = mybir.dt.int16

NCORES = 8
NSEG = 4          # table segments (2 cores' rows each) so idx fits int16


def _wrap16(a):
    """[128, n] per-slot idx array -> dma_gather idx layout [128, n*8] int16.

    Slot (chunk c, partition p) maps to idx-list position i = c*128 + p;
    the ucode reads position i from [p16=i%16, col=c*8 + (i%128)//16],
    partitions replicated x8.
    """
    p128, n = a.shape
    assert p128 == 128
    m = np.zeros((16, n, 8), np.int16)
    for s16 in range(8):
        m[:, :, s16] = a[s16 * 16:(s16 + 1) * 16, :]
    m = m.reshape(16, n * 8)
    return np.tile(m, (8, 1)).copy()


class Dims:
    def __init__(self, N, E, P, FIN, H, D1, CS, D, PG):
        self.N, self.E, self.P = N, E, P
        self.FIN, self.H, self.D1 = FIN, H, D1
        self.CS = list(CS)             # chunks per window per segment plane
        self.D = D                     # max node in-degree
        self.PG = PG                   # decoder chunks per pair-group (even)
        self.NSH = N // NCORES
        self.W = (self.NSH + 127) // 128
        self.NPAD = self.W * 128
        self.SEGR = 2 * self.NPAD      # rows per table segment
        self.NCH = self.W * sum(CS)    # total chunk columns
        self.PB = [self.W * sum(CS[:s]) for s in range(NSEG)]  # plane col base
        self.WD = self.W * D
        self.PPC = P // NCORES
        self.PCH = self.PPC // 128     # real decoder chunks
        self.PCH2 = 16 * PG            # padded decoder chunks (16 groups)
        self.KF = FIN // 128
        self.GBW = 1   # >1 risks SWDGE ring/packet overflow (561-desc calls hang HW)
        self.WH = 2 if self.W % 2 == 0 else 1
        assert self.SEGR <= 32768
        assert PG % 2 == 0


def chunk_params(edge_index, idx, N):
    """Uniform (CS per segment, D, PG) from the data."""
    NSH = N // NCORES
    W = (NSH + 127) // 128
    NPAD = W * 128
    col = np.asarray(edge_index[1], np.int64)
    row = np.asarray(edge_index[0], np.int64)
    order = np.argsort(col, kind="stable")
    col_s, row_s = col[order], row[order]
    D = int(np.bincount(col_s, minlength=N).max())
    seg_of_row = ((row_s // NSH) * NPAD + (row_s % NSH)) // (2 * NPAD)
    CS = [1] * NSEG
    for c in range(NCORES):
        lo, hi = np.searchsorted(col_s, [c * NSH, (c + 1) * NSH])
        wof = (col_s[lo:hi] - c * NSH) >> 7
        sof = seg_of_row[lo:hi]
        cnt = np.bincount(wof * NSEG + sof, minlength=W * NSEG).reshape(W, NSEG)
        for s in range(NSEG):
            CS[s] = max(CS[s], int(np.ceil(cnt[:, s].max() / 128)))
    # decoder groups
    i1 = np.asarray(idx[0], np.int64)
    i2 = np.asarray(idx[1], np.int64)
    g1 = ((i1 // NSH) * NPAD + (i1 % NSH)) // (2 * NPAD)
    g2 = ((i2 // NSH) * NPAD + (i2 % NSH)) // (2 * NPAD)
    PPC = i1.shape[0] // NCORES
    PG = 1
    for c in range(NCORES):
        g = g1[c * PPC:(c + 1) * PPC] * NSEG + g2[c * PPC:(c + 1) * PPC]
        cnt = np.bincount(g, minlength=NSEG * NSEG)
        PG = max(PG, int(np.ceil(cnt.max() / 128)))
    if PG % 2:
        PG += 1
    return CS, D, PG


def preprocess(d, x, curvature, edge_index, idx, b_gcn, lin1_w, lin1_b):
    """Index plumbing: sort/bin/pad edges and pairs, build per-core inputs."""
    row = np.asarray(edge_index[0], dtype=np.int64)
    col = np.asarray(edge_index[1], dtype=np.int64)
    cur = np.asarray(curvature, dtype=np.float32)
    order = np.argsort(col, kind="stable")
    row_s, col_s, cur_s = row[order], col[order], cur[order]
    bounds = np.searchsorted(col_s, np.arange(NCORES + 1) * d.NSH)

    iota_f = np.broadcast_to(np.arange(128, dtype=np.float32), (128, 128)).copy()
    b_mat = np.broadcast_to(np.asarray(b_gcn, np.float32)[None, :], (128, d.H)).copy()
    linw_rep = np.broadcast_to(np.asarray(lin1_w, np.float32)[:, 0][None, :], (128, 10)).copy()
    linb_rep = np.full((128, 1), np.float32(np.asarray(lin1_b).reshape(-1)[0]), np.float32)

    def gidx(nodes):
        return ((nodes // d.NSH) * d.NPAD + (nodes % d.NSH)).astype(np.int64)

    maps = []
    perms = []
    for c in range(NCORES):
        lo, hi = bounds[c], bounds[c + 1]
        colr = (col_s[lo:hi] - c * d.NSH).astype(np.int64)
        grow = gidx(row_s[lo:hi])
        curc = cur_s[lo:hi]
        n_c = colr.shape[0]
        wof = colr >> 7
        sof = grow // d.SEGR
        # order edges by (window, segment) [stable within = by col]
        okey = np.argsort(wof * NSEG + sof, kind="stable")
        colr, grow, curc = colr[okey], grow[okey], curc[okey]
        wof, sof = wof[okey], sof[okey]
        # chunk column for each edge
        cnt = np.bincount(wof * NSEG + sof, minlength=d.W * NSEG).reshape(d.W, NSEG)
        start = np.zeros(d.W * NSEG, np.int64)
        start[1:] = np.cumsum(cnt.reshape(-1))[:-1]
        pos = np.arange(n_c) - start[wof * NSEG + sof]
        ci = np.array(d.PB, np.int64)[sof] + wof * np.array(d.CS, np.int64)[sof] + (pos >> 7)
        p = (pos & 127).astype(np.int64)

        dstf = np.full((128, d.NCH), -1.0, np.float32)
        dstf[p, ci] = (colr & 127).astype(np.float32)
        curb = np.zeros((128, d.NCH), np.float32)
        curb[p, ci] = curc
        ridx = np.zeros((128, d.NCH), np.int64)
        ridx[p, ci] = grow - sof * d.SEGR
        g16 = _wrap16(ridx.astype(np.int16))

        # degree layout (per-node padded)
        colr2 = col_s[lo:hi] - c * d.NSH
        cur2 = cur_s[lo:hi]
        nstart = np.searchsorted(colr2, np.arange(d.NSH))
        dpos = np.arange(n_c) - nstart[colr2]
        pn = (colr2 & 127).astype(np.int64)
        fi = (colr2 >> 7) * d.D + dpos
        curd = np.zeros((128, d.WD), np.float32)
        curd[pn, fi] = cur2
        maskd = np.zeros((128, d.WD), np.float32)
        maskd[pn, fi] = 1.0

        # host-transposed x shard, bf16: xt[p, k*NPAD + n] = x[base+n, k*128+p]
        x_pad = np.zeros((d.NPAD, d.FIN), np.float32)
        x_pad[:d.NSH] = x[c * d.NSH:(c + 1) * d.NSH]
        import ml_dtypes
        xt = np.ascontiguousarray(
            x_pad.reshape(d.NPAD, d.KF, 128).transpose(2, 1, 0)
        ).reshape(128, d.KF * d.NPAD).astype(ml_dtypes.bfloat16)

        # decoder pair groups
        gi1 = gidx(np.asarray(idx[0][c * d.PPC:(c + 1) * d.PPC], np.int64))
        gi2 = gidx(np.asarray(idx[1][c * d.PPC:(c + 1) * d.PPC], np.int64))
        grp = (gi1 // d.SEGR) * NSEG + (gi2 // d.SEGR)
        pkey = np.argsort(grp, kind="stable")
        gcnt = np.bincount(grp, minlength=16)
        gstart = np.zeros(16, np.int64)
        gstart[1:] = np.cumsum(gcnt)[:-1]
        ppos = np.arange(d.PPC) - gstart[grp[pkey]]
        slot = grp[pkey] * d.PG * 128 + ppos        # slot in padded layout
        o1a = np.zeros((128, d.PCH2), np.int64)
        o2a = np.zeros((128, d.PCH2), np.int64)
        o1a[slot & 127, slot >> 7] = gi1[pkey] % d.SEGR
        o2a[slot & 127, slot >> 7] = gi2[pkey] % d.SEGR
        o1g = _wrap16(o1a.astype(np.int16))
        o2g = _wrap16(o2a.astype(np.int16))
        perms.append(np.asarray(slot))              # real pair pkey[j] -> slot[j]
        pk = np.empty(d.PPC, np.int64)
        pk[:] = pkey
        perms[-1] = (pk, np.asarray(slot))

        maps.append(dict(
            xt=xt, curb=curb, dstf=dstf, g16=g16,
            curd=curd, maskd=maskd, o1g=o1g, o2g=o2g,
            iota_f=iota_f, b_mat=b_mat, linw=linw_rep, linb=linb_rep,
        ))
    return maps, perms


def build(d):
    nc = bacc.Bacc("TRN2", target_bir_lowering=False, debug=False,
                   num_devices=NCORES)
    H, D, W = d.H, d.D, d.W

    xt_d = nc.dram_tensor("xt", [128, d.KF * d.NPAD], BF16, kind="ExternalInput")
    curb = nc.dram_tensor("curb", [128, d.NCH], F32, kind="ExternalInput")
    dstf = nc.dram_tensor("dstf", [128, d.NCH], F32, kind="ExternalInput")
    g16 = nc.dram_tensor("g16", [128, d.NCH * 8], I16, kind="ExternalInput")
    curd = nc.dram_tensor("curd", [128, d.WD], F32, kind="ExternalInput")
    maskd = nc.dram_tensor("maskd", [128, d.WD], F32, kind="ExternalInput")
    o1g = nc.dram_tensor("o1g", [128, d.PCH2 * 8], I16, kind="ExternalInput")
    o2g = nc.dram_tensor("o2g", [128, d.PCH2 * 8], I16, kind="ExternalInput")
    iota_f = nc.dram_tensor("iota_f", [128, 128], F32, kind="ExternalInput")
    b_mat = nc.dram_tensor("b_mat", [128, H], F32, kind="ExternalInput")
    linw = nc.dram_tensor("linw", [128, 10], F32, kind="ExternalInput")
    linb = nc.dram_tensor("linb", [128, 1], F32, kind="ExternalInput")
    w_gcn = nc.dram_tensor("w_gcn", [d.FIN, H], F32, kind="ExternalInput")
    dec1_w = nc.dram_tensor("dec1_w", [4 * H, d.D1], F32, kind="ExternalInput")
    dec1_b = nc.dram_tensor("dec1_b", [d.D1], F32, kind="ExternalInput")
    dec2_w = nc.dram_tensor("dec2_w", [d.D1, 1], F32, kind="ExternalInput")
    dec2_b = nc.dram_tensor("dec2_b", [1], F32, kind="ExternalInput")
    out_d = nc.dram_tensor("out_d", [d.PCH2 * 128, 1], F32, kind="ExternalOutput")

    AT = mybir.ActivationFunctionType
    OP = mybir.AluOpType
    MB = d.D1 // 128

    with ExitStack() as ctx:
        tc = ctx.enter_context(tile.TileContext(nc))
        const = ctx.enter_context(tc.tile_pool(name="const", bufs=1))
        sb = ctx.enter_context(tc.tile_pool(name="sb", bufs=3))
        big = ctx.enter_context(tc.tile_pool(name="big", bufs=3))
        gp = ctx.enter_context(tc.tile_pool(name="gp", bufs=5))   # gather tiles
        ps = ctx.enter_context(tc.tile_pool(name="ps", bufs=2, space="PSUM"))
        dr = ctx.enter_context(tc.tile_pool(name="dr", bufs=1, space="DRAM"))

        # ---------- constants ----------
        iota_sb = const.tile([128, 128], F32, tag="iota32")
        nc.sync.dma_start(out=iota_sb[:], in_=iota_f.ap())
        iota_bf = const.tile([128, 128], BF16, tag="iotabf")
        nc.vector.tensor_copy(out=iota_bf[:], in_=iota_sb[:])
        ident = const.tile([128, 128], BF16, tag="ident")
        make_identity(nc, ident[:])
        ident_f = const.tile([128, 128], F32, tag="identf")
        make_identity(nc, ident_f[:])
        bmat_sb = const.tile([128, H], F32, tag="bmat")
        nc.sync.dma_start(out=bmat_sb[:], in_=b_mat.ap())

        linw_sb = const.tile([128, 10], F32, tag="linw")
        nc.sync.dma_start(out=linw_sb[:], in_=linw.ap())
        linb_sb = const.tile([128, 1], F32, tag="linb")
        nc.sync.dma_start(out=linb_sb[:], in_=linb.ap())
        coef = const.tile([128, 10], F32, tag="coef")
        nc.vector.tensor_scalar_mul(coef[:], linw_sb[:], 0.5)
        csum = const.tile([128, 1], F32, tag="csum")
        nc.vector.tensor_reduce(out=csum[:], in_=coef[:], axis=mybir.AxisListType.X,
                                op=OP.add)
        cconst = const.tile([128, 1], F32, tag="cconst")
        nc.vector.tensor_add(cconst[:], csum[:], linb_sb[:])

        wg_sb = const.tile([128, d.KF, H], F32, tag="wg32")
        nc.sync.dma_start(out=wg_sb[:], in_=w_gcn.ap().rearrange("(k p) h -> p k h", p=128))
        wgr = const.tile([128, d.KF, H], BF16, tag="wgr")
        nc.vector.tensor_copy(out=wgr[:], in_=wg_sb[:])

        d1_sb = big.tile([128, 4, d.D1], F32, tag="big")
        nc.sync.dma_start(out=d1_sb[:], in_=dec1_w.ap().rearrange("(b p) d -> p b d", p=128))
        wc_sb = const.tile([128, 3, d.D1], BF16, tag="wcf")
        nc.vector.tensor_add(wc_sb[:, 0, :], d1_sb[:, 0, :], d1_sb[:, 2, :])
        nc.vector.tensor_add(wc_sb[:, 1, :], d1_sb[:, 0, :], d1_sb[:, 3, :])
        nc.vector.tensor_copy(out=wc_sb[:, 2, :], in_=d1_sb[:, 1, :])

        d1b_sb = const.tile([128, MB], F32, tag="d1b")
        nc.sync.dma_start(out=d1b_sb[:], in_=dec1_b.ap().rearrange("(b p) -> p b", p=128))
        d2_sb = const.tile([128, MB, 1], F32, tag="d232")
        nc.sync.dma_start(out=d2_sb[:], in_=dec2_w.ap().rearrange("(b p) o -> p b o", p=128))
        d2r = const.tile([128, MB, 1], BF16, tag="d2r")
        nc.vector.tensor_copy(out=d2r[:], in_=d2_sb[:])
        d2b_sb = const.tile([1, 1], F32, tag="d2b")
        nc.sync.dma_start(out=d2b_sb[:], in_=dec2_b.ap()[:, None])

        # ---------- edge weights (chunk layout) ----------
        def horner(src_ap, n):
            t = big.tile([128, n], F32, tag="big")
            nc.scalar.activation(out=t[:], in_=src_ap, func=AT.Exp, scale=-1.0)
            acc = big.tile([128, n], F32, tag="big")
            nc.vector.tensor_scalar_mul(acc[:], t[:], coef[:, 9:10])
            for k in range(8, -1, -1):
                nc.vector.scalar_tensor_tensor(
                    out=acc[:], in0=acc[:], scalar=coef[:, k:k + 1], in1=t[:],
                    op0=OP.add, op1=OP.mult)
            nc.vector.tensor_scalar_add(acc[:], acc[:], cconst[:])
            return acc

        # ---------- degrees ----------
        WHF = d.WD // d.WH
        WHW = W // d.WH
        deg_r = const.tile([128, W], F32, tag="deg")
        for h in range(d.WH):
            cu = big.tile([128, WHF], F32, tag="big")
            nc.sync.dma_start(out=cu[:], in_=curd.ap()[:, h * WHF:(h + 1) * WHF])
            ewd = horner(cu[:], WHF)
            mk = big.tile([128, WHF], F32, tag="big")
            nc.sync.dma_start(out=mk[:], in_=maskd.ap()[:, h * WHF:(h + 1) * WHF])
            nc.vector.tensor_mul(ewd[:], ewd[:], mk[:])
            nc.vector.tensor_reduce(
                out=deg_r[:, h * WHW:(h + 1) * WHW],
                in_=ewd[:].rearrange("p (w dd) -> p w dd", dd=D),
                axis=mybir.AxisListType.X, op=OP.add)
        nc.vector.tensor_scalar_add(deg_r[:], deg_r[:], 1.0)
        mw = const.tile([128, W], F32, tag="mw")
        nc.vector.tensor_single_scalar(out=mw[:], in_=deg_r[:], scalar=0.0, op=OP.is_gt)
        degm = const.tile([128, W], F32, tag="degm")
        nc.vector.tensor_mul(degm[:], deg_r[:], mw[:])
        onem = const.tile([128, W], F32, tag="onem")
        nc.vector.tensor_scalar(out=onem[:], in0=mw[:], scalar1=-1.0, scalar2=1.0,
                                op0=OP.mult, op1=OP.add)
        nc.vector.tensor_add(degm[:], degm[:], onem[:])
        rec = const.tile([128, W], F32, tag="rec")
        nc.vector.reciprocal(out=rec[:], in_=degm[:])
        dsq = const.tile([128, W], F32, tag="dsq")
        nc.scalar.activation(out=dsq[:], in_=rec[:], func=AT.Sqrt)
        dinv = const.tile([128, W], F32, tag="dinv")
        nc.vector.tensor_mul(dinv[:], dsq[:], mw[:])

        # ---------- xw + y per window ----------
        y_loc = dr.tile([d.NPAD, H], BF16)
        y_loc_r = y_loc[:].rearrange("(w p) h -> p w h", p=128)
        xt_r = xt_d.ap().rearrange("p (k n) -> p k n", k=d.KF)
        GX = 4
        for wb in range((W + GX - 1) // GX):
            w0 = wb * GX
            gw = min(GX, W - w0)
            xc = sb.tile([128, d.KF, GX * 128], BF16, tag="xc", bufs=2)
            nc.sync.dma_start(out=xc[:, :, :gw * 128],
                              in_=xt_r[:, :, w0 * 128:(w0 + gw) * 128])
            for wi in range(gw):
                w = w0 + wi
                pxw = ps.tile([128, H], F32, tag="win")
                for k in range(d.KF):
                    nc.tensor.matmul(pxw[:],
                                     lhsT=xc[:, k, wi * 128:(wi + 1) * 128],
                                     rhs=wgr[:, k, :],
                                     start=(k == 0), stop=(k == d.KF - 1))
                yw = sb.tile([128, H], BF16, tag="yw")
                nc.scalar.activation(out=yw[:], in_=pxw[:], func=AT.Copy,
                                     scale=dinv[:, w:w + 1])
                nc.sync.dma_start(out=y_loc_r[:, w, :], in_=yw[:])

        y_full = dr.tile([NCORES * d.NPAD, H], BF16, addr_space="Shared")
        nc.gpsimd.collective_compute(
            "AllGather", OP.bypass, replica_groups=[list(range(NCORES))],
            ins=[y_loc[:]], outs=[y_full[:]])

        curb_sb = big.tile([128, d.NCH], F32, tag="big")
        nc.sync.dma_start(out=curb_sb[:], in_=curb.ap())
        ew_nch = horner(curb_sb[:], d.NCH)
        ew_f = const.tile([128, d.NCH], BF16, tag="ewf")
        nc.vector.tensor_copy(out=ew_f[:], in_=ew_nch[:])

        dst_sb = big.tile([128, d.NCH], F32, tag="big")
        nc.sync.dma_start(out=dst_sb[:], in_=dstf.ap())
        dst_bf = const.tile([128, d.NCH], BF16, tag="dstbf")
        nc.vector.tensor_copy(out=dst_bf[:], in_=dst_sb[:])

        # ---------- aggregation ----------
        # Window groups of G: one big gather per (segment plane, group), one
        # batched one-hot build + in-place ew fold per (plane, group), then
        # per-window matmul chains accumulating all 4 planes into psum.
        x1_loc = dr.tile([d.NPAD, H], BF16)
        x1_loc_r = x1_loc[:].rearrange("(w p) h -> p w h", p=128)
        G = 4
        for gb in range((W + G - 1) // G):
            w0 = gb * G
            gw = min(G, W - w0)
            yts, sss = [], []
            for s in range(NSEG):
                cs = d.CS[s]
                n_idx = gw * cs * 128
                ix = sb.tile([128, G * cs * 8], I16, tag="ix")
                c0 = d.PB[s] + w0 * cs
                nc.sync.dma_start(out=ix[:, :gw * cs * 8],
                                  in_=g16.ap()[:, c0 * 8:(c0 + gw * cs) * 8])
                yt = gp.tile([128, G * cs, H], BF16, tag="yt", bufs=5)
                nc.gpsimd.dma_gather(
                    out_ap=yt[:, :gw * cs, :],
                    in_ap=y_full[s * d.SEGR:(s + 1) * d.SEGR, :],
                    idxs_ap=ix[:, :gw * cs * 8], num_idxs=n_idx,
                    num_idxs_reg=n_idx, elem_size=H, single_packet=False)
                ss = gp.tile([128, G * cs, 128], BF16, tag="sc", bufs=4)
                nc.vector.tensor_tensor(
                    out=ss[:, :gw * cs, :],
                    in0=dst_bf[:, c0:c0 + gw * cs, None].to_broadcast(
                        [128, gw * cs, 128]),
                    in1=iota_bf[:, None, :].to_broadcast([128, gw * cs, 128]),
                    op=OP.is_equal)
                nc.vector.tensor_tensor(
                    out=ss[:, :gw * cs, :], in0=ss[:, :gw * cs, :],
                    in1=ew_f[:, c0:c0 + gw * cs, None].to_broadcast(
                        [128, gw * cs, 128]),
                    op=OP.mult)
                yts.append(yt)
                sss.append(ss)
            for wi in range(gw):
                w = w0 + wi
                pw = ps.tile([128, H], F32, tag="win")
                first = True
                for s in range(NSEG):
                    cs = d.CS[s]
                    for k in range(cs):
                        j = wi * cs + k
                        last = (s == NSEG - 1) and (k == cs - 1)
                        nc.tensor.matmul(pw[:], lhsT=sss[s][:, j, :],
                                         rhs=yts[s][:, j, :],
                                         start=first, stop=last)
                        first = False
                ywr = sb.tile([128, H], BF16, tag="ywr")
                nc.sync.dma_start(out=ywr[:], in_=y_loc_r[:, w, :])
                t1 = sb.tile([128, H], F32, tag="t1")
                nc.vector.tensor_add(t1[:], pw[:], ywr[:])
                t2 = sb.tile([128, H], F32, tag="t2")
                nc.vector.scalar_tensor_tensor(
                    out=t2[:], in0=t1[:], scalar=dinv[:, w:w + 1], in1=bmat_sb[:],
                    op0=OP.mult, op1=OP.add)
                x1w = sb.tile([128, H], BF16, tag="x1w")
                nc.scalar.activation(out=x1w[:], in_=t2[:], func=AT.Relu)
                nc.sync.dma_start(out=x1_loc_r[:, w, :], in_=x1w[:])

        x1_full = dr.tile([NCORES * d.NPAD, H], BF16, addr_space="Shared")
        nc.gpsimd.collective_compute(
            "AllGather", OP.bypass, replica_groups=[list(range(NCORES))],
            ins=[x1_loc[:]], outs=[x1_full[:]])

        # ---------- pair decoder ----------
        PG = d.PG
        NT2 = 2 * PG // 4               # 512-pair tiles per super-batch
        out_r = out_d.ap().rearrange("a b -> b a")      # [1, PCH2*128]
        for sbch in range(8):            # super-batches of 2 groups
            g0 = sbch * 2
            e1 = gp.tile([128, 2 * PG, H], BF16, tag="et", bufs=5)
            e2 = gp.tile([128, 2 * PG, H], BF16, tag="et", bufs=5)
            for gi in range(2):
                g = g0 + gi
                a_seg, b_seg = g // NSEG, g % NSEG
                for (tile_, src_seg, arr) in ((e1, a_seg, o1g), (e2, b_seg, o2g)):
                    c0 = g * PG
                    ix = sb.tile([128, PG * 8], I16, tag="ixd")
                    nc.sync.dma_start(out=ix[:], in_=arr.ap()[:, c0 * 8:(c0 + PG) * 8])
                    nc.gpsimd.dma_gather(
                        out_ap=tile_[:, gi * PG:(gi + 1) * PG, :],
                        in_ap=x1_full[src_seg * d.SEGR:(src_seg + 1) * d.SEGR, :],
                        idxs_ap=ix[:], num_idxs=PG * 128, num_idxs_reg=PG * 128,
                        elem_size=H, single_packet=False)
            em = gp.tile([128, 2 * PG, H], BF16, tag="et", bufs=5)
            nc.vector.tensor_mul(em[:], e1[:], e2[:])
            ob = sb.tile([1, NT2 * 512], F32, tag="ob")
            for nt in range(NT2):
                cT = sb.tile([128, 3, 4, 128], BF16, tag="cT")
                for jj in range(4):
                    j = nt * 4 + jj
                    for cix, srct in enumerate((e1, e2, em)):
                        pt = ps.tile([128, 128], BF16, tag="trf")
                        nc.tensor.transpose(pt[:], srct[:, j, :], ident[:])
                        if (j + cix) % 2 == 0:
                            nc.scalar.copy(out=cT[:, cix, jj, :], in_=pt[:])
                        else:
                            nc.vector.tensor_copy(out=cT[:, cix, jj, :], in_=pt[:])
                h_sb = sb.tile([128, MB, 512], BF16, tag="hsb")
                for mb in range(MB):
                    ph = ps.tile([128, 512], F32, tag="ph")
                    for cix in range(3):
                        nc.tensor.matmul(
                            ph[:], lhsT=wc_sb[:, cix, mb * 128:(mb + 1) * 128],
                            rhs=cT[:, cix, :, :],
                            start=(cix == 0), stop=(cix == 2))
                    nc.scalar.activation(out=h_sb[:, mb, :], in_=ph[:], func=AT.Relu,
                                         bias=d1b_sb[:, mb:mb + 1])
                po = ps.tile([1, 512], F32, tag="po")
                for mb in range(MB):
                    nc.tensor.matmul(po[:], lhsT=d2r[:, mb, :],
                                     rhs=h_sb[:, mb, :],
                                     start=(mb == 0), stop=(mb == MB - 1))
                nc.scalar.activation(out=ob[:, nt * 512:(nt + 1) * 512], in_=po[:],
                                     func=AT.Identity, bias=d2b_sb[:, :])
            off = sbch * NT2 * 512
            nc.sync.dma_start(out=out_r[:, off:off + NT2 * 512], in_=ob[:])

    nc.compile()
    return nc


_CACHE = {}
TRACE = False          # test harness sets True to capture NTFF profile
LAST_RESULT = None     # BassKernelResults of the most recent run


def kernel(**inputs):
    x = np.asarray(inputs["x"], np.float32)
    curvature = np.asarray(inputs["curvature"], np.float32)
    edge_index = np.asarray(inputs["edge_index"])
    idx = np.asarray(inputs["idx"])
    N, FIN = x.shape
    E = edge_index.shape[1]
    P = idx.shape[1]
    H = np.asarray(inputs["W_gcn"]).shape[1]
    D1 = np.asarray(inputs["dec1_w"]).shape[1]

    CS, D, PG = chunk_params(edge_index, idx, N)
    d = Dims(N, E, P, FIN, H, D1, CS, D, PG)
    maps, perms = preprocess(d, x, curvature, edge_index, idx,
                             inputs["b_gcn"], inputs["lin1_w"], inputs["lin1_b"])
    shared = dict(
        w_gcn=np.asarray(inputs["W_gcn"], np.float32),
        dec1_w=np.asarray(inputs["dec1_w"], np.float32),
        dec1_b=np.asarray(inputs["dec1_b"], np.float32).reshape(-1),
        dec2_w=np.asarray(inputs["dec2_w"], np.float32),
        dec2_b=np.asarray(inputs["dec2_b"], np.float32).reshape(-1),
    )
    for m in maps:
        m.update(shared)

    key = (N, E, P, tuple(CS), D, PG)
    if key not in _CACHE:
        _CACHE[key] = build(d)
    nc = _CACHE[key]

    from concourse.bass_utils import run_bass_kernel_spmd
    res = run_bass_kernel_spmd(nc, maps, core_ids=list(range(NCORES)),
                               trace=TRACE)
    global LAST_RESULT
    LAST_RESULT = res
    out = np.empty((P, 1), np.float32)
    for c in range(NCORES):
        vals = np.asarray(res.results[c]["out_d"], np.float32)[:, 0]
        pk, slot = perms[c]
        out[c * d.PPC + pk, 0] = vals[slot]
    return out

